# revision 8
# baseline (speedup 1.0000x reference)
"""Trainium2 Bass kernel for nn_Block_58497454571919 (dense transformer block).

Reference semantics (B=4, S=2048, D=2048, H=16, Dh=128, DFF=8192):
  X = x @ W1.T + b1 ; Q,K,V = split(X)
  per (b,h): scores[d,e] = sum_s Q[b,s,hd]K[b,s,he] / sqrt(S)  (feature-attention)
             w = softmax(scores, axis=e);  out[d,s] = sum_e w[d,e] V[b,s,he]
  attn_pre[b, h*128+d, s] = out[d,s]   (raw memory reshape)
  a = attn_pre @ W2.T + b2 ; t1 = a + x ; x1 = global_scalar_LN(t1, lnw1, lnb1)
  m = gelu_tanh(x1 @ fc.T + fcb) @ proj.T + projb ; t2 = m + x1
  y = global_scalar_LN(t2, lnw2, lnb2)

Distribution over 8 cores: core c owns heads {2c, 2c+1} == output rows
[256c, 256c+256) of every batch. The QKV projection for those heads needs all
tokens (full x), W2/LN/FFN are row-parallel on the core's 4*256=1024 rows.
The only cross-core data dependency is the global-scalar LayerNorm mean/var:
two tiny AllReduces of (sum, sumsq).

On-device layouts (all "transposed" so no device transposes are needed):
  QK[b]   [128 s_in, 16 s_out, 512 (q 256|k 256)] bf16
  VT[b]   [128 vf_in, 2 head, 2048 s]             bf16
  attnT   [128 s_in, 16 s_out, 1024 i]            bf16   i = b*256 + hl*128 + d
  t1T/x1T [128 n_in, 16 n_out, 1024 i]            f32/bf16
"""
import math
import os
import sys
import types

import numpy as np
import ml_dtypes

import concourse.bass as bass
import concourse.bacc as bacc
import concourse.mybir as mybir
import concourse.tile as tile
from concourse import bass_utils
from concourse.masks import make_identity

F32 = mybir.dt.float32
BF16 = mybir.dt.bfloat16
AF = mybir.ActivationFunctionType
OP = mybir.AluOpType

N_CORES = 8
B, S, D, H, DH, DFF = 4, 2048, 2048, 16, 128, 8192
P = 128
EPS = 1e-12
SM_SCALE = 1.0 / math.sqrt(S)
N_GLOB = float(B * S * D)          # 16777216 elements in each layernorm
NP_PART = 32 * 512                 # elements per partition per core in stats
N_PGROUPS = 1024.0                 # 8 cores * 128 partitions

TRACE = False          # set by test.py to capture an NTFF profile
LAST_RESULT = None     # BassKernelResults stash for test.py


def _register_ntff_hook():
    """The agent image's antenv lacks axon_hooks; inject it so trace=True works."""
    if "antenv.axon_hooks" in sys.modules:
        return
    mod = types.ModuleType("antenv.axon_hooks")
    mod._hook = None
    mod.set_axon_ntff_profile_hook = lambda h: setattr(mod, "_hook", h)
    mod.get_axon_ntff_profile_hook = lambda: mod._hook
    sys.modules["antenv.axon_hooks"] = mod
    import antenv

    antenv.axon_hooks = mod
    try:
        from trn_agent_boot.trn_boot import _ntff_profile_via_ctypes

        mod.set_axon_ntff_profile_hook(
            _ntff_profile_via_ctypes("/opt/axon/libaxon_pjrt.so")
        )
    except Exception:
        pass


def build_program():
    nc = bacc.Bacc("TRN2", target_bir_lowering=False, debug=False, num_devices=N_CORES)

    def din(name, shape, dtype):
        return nc.dram_tensor(name, shape, dtype, kind="ExternalInput").ap()

    xq = din("xq", [B, 4, P, 16, 512], BF16)        # x^T tiles: [b, sb, d_in, d_out, s]
    w1qk = din("w1qk", [P, 16, 512], BF16)          # [d_in, d_out, (q|k) feat] (core slice)
    b1qk = din("b1qk", [P, 512], F32)               # replicated over partitions
    w1v = din("w1v", [P, 16, 256], BF16)            # [d_in, d_out, vfeat]
    b1v = din("b1v", [P, 2], F32)                   # [vf_in, head]
    w2 = din("w2", [16, P, 16, 128], BF16)          # [n_blk, s_in, s_out, n]
    b2 = din("b2", [P, 16], F32)                    # [n_in, n_out]
    xres = din("xres", [P, 16, 1024], F32)          # x^T rows slice  [n_in, n_out, i]
    fc = din("fc", [64, P, 16, 128], BF16)          # [f_blk, d_in, d_out, f]
    fcb = din("fcb", [P, 64], F32)                  # [f_in, f_blk]
    proj = din("proj", [16, P, 64, 128], BF16)      # [n_blk, f_in, f_out, n]
    projb = din("projb", [P, 16], F32)              # [n_in, n_out]
    lnw1 = din("lnw1", [P, 16], F32)
    lnb1 = din("lnb1", [P, 16], F32)
    lnw2 = din("lnw2", [P, 16], F32)
    lnb2 = din("lnb2", [P, 16], F32)
    y_out = nc.dram_tensor("y", [P, 16, 1024], F32, kind="ExternalOutput").ap()

    with tile.TileContext(nc) as tc:
        _emit(nc, tc, xq, w1qk, b1qk, w1v, b1v, w2, b2, xres, fc, fcb, proj,
              projb, lnw1, lnb1, lnw2, lnb2, y_out)
    nc.compile()
    return nc


def _emit(nc, tc, xq, w1qk, b1qk, w1v, b1v, w2, b2, xres, fc, fcb, proj,
          projb, lnw1, lnb1, lnw2, lnb2, y_out):
    with (
        tc.tile_pool(name="consts", bufs=1) as consts,
        tc.tile_pool(name="stats", bufs=1) as stats,
        tc.tile_pool(name="small", bufs=3) as small,
        tc.tile_pool(name="dram", bufs=1, space="DRAM") as dram,
        tc.tile_pool(name="ps_big", bufs=4, space="PSUM") as ps_big,
        tc.tile_pool(name="ps_sm", bufs=3, space="PSUM") as ps_sm,
        tc.tile_pool(name="ps_red", bufs=1, space="PSUM") as ps_red,
        tc.tile_pool(name="x1pool", bufs=1) as x1_pool,
    ):
        # ---- long-lived constants ----
        b2_sb = consts.tile([P, 16], F32, name="b2_sb")
        nc.sync.dma_start(b2_sb[:], b2[:])
        fcb_sb = consts.tile([P, 64], F32, name="fcb_sb")
        nc.sync.dma_start(fcb_sb[:], fcb[:])
        projb_sb = consts.tile([P, 16], F32, name="projb_sb")
        nc.sync.dma_start(projb_sb[:], projb[:])
        lnw1_sb = consts.tile([P, 16], F32, name="lnw1_sb")
        nc.sync.dma_start(lnw1_sb[:], lnw1[:])
        lnb1_sb = consts.tile([P, 16], F32, name="lnb1_sb")
        nc.sync.dma_start(lnb1_sb[:], lnb1[:])
        lnw2_sb = consts.tile([P, 16], F32, name="lnw2_sb")
        nc.sync.dma_start(lnw2_sb[:], lnw2[:])
        lnb2_sb = consts.tile([P, 16], F32, name="lnb2_sb")
        nc.sync.dma_start(lnb2_sb[:], lnb2[:])
        ident = consts.tile([P, P], BF16, name="ident")
        make_identity(nc, ident[:])
        ones = consts.tile([P, 1], F32, name="ones")
        nc.vector.memset(ones[:], 1.0)
        epsb = consts.tile([P, 1], F32, name="epsb")
        nc.vector.memset(epsb[:], EPS)

        stats1 = stats.tile([P, 32, 6], F32, name="stats1")
        stats2 = stats.tile([P, 32, 6], F32, name="stats2")
        x1T = x1_pool.tile([P, 16, 1024], BF16, name="x1T")

        with tc.tile_pool(name="attn", bufs=1) as attn_pool:
            attnT = attn_pool.tile([P, 16, 1024], BF16, name="attnT")

            # ============ Phases 1-2: QKV projection + feature-attention ============
            with (
                tc.tile_pool(name="w1pool", bufs=1) as w1_pool,
                tc.tile_pool(name="xq", bufs=2) as xq_pool,
                tc.tile_pool(name="qkpool", bufs=2) as qk_pool,
            ):
                w1qk_sb = w1_pool.tile([P, 16, 512], BF16, name="w1qk_sb")
                nc.sync.dma_start(w1qk_sb[:], w1qk[:])
                b1qk_sb = w1_pool.tile([P, 512], F32, name="b1qk_sb")
                nc.sync.dma_start(b1qk_sb[:], b1qk[:])
                w1v_sb = w1_pool.tile([P, 16, 256], BF16, name="w1v_sb")
                nc.sync.dma_start(w1v_sb[:], w1v[:])
                b1v_sb = w1_pool.tile([P, 2], F32, name="b1v_sb")
                nc.sync.dma_start(b1v_sb[:], b1v[:])

                for b in range(B):
                    QK = qk_pool.tile([P, 16, 512], BF16, name="QK", tag="QK")
                    VT = qk_pool.tile([P, 2, S], BF16, name="VT", tag="VT")
                    # --- phase 1: QK and V^T for batch b ---
                    for sb in range(4):
                        xt = xq_pool.tile([P, 16, 512], BF16, name="xt", tag="xt")
                        nc.sync.dma_start(xt[:], xq[b, sb])
                        for ss in range(4):  # 128-token blocks within the 512 chunk
                            pqk = ps_big.tile([P, 512], F32, name="pqk", tag="psbig")
                            for do in range(16):
                                nc.tensor.matmul(
                                    pqk[:], xt[:, do, ss * 128:(ss + 1) * 128],
                                    w1qk_sb[:, do, :], start=(do == 0), stop=(do == 15),
                                )
                            nc.vector.tensor_tensor(
                                QK[:, sb * 4 + ss, :], pqk[:], b1qk_sb[:], OP.add)
                        for vo in range(2):
                            pv = ps_big.tile([P, 512], F32, name="pv", tag="psbig")
                            for do in range(16):
                                nc.tensor.matmul(
                                    pv[:], w1v_sb[:, do, vo * 128:(vo + 1) * 128],
                                    xt[:, do, :], start=(do == 0), stop=(do == 15),
                                )
                            nc.vector.tensor_scalar(
                                VT[:, vo, sb * 512:(sb + 1) * 512], pv[:],
                                b1v_sb[:, vo:vo + 1], None, OP.add)
                    # --- phase 2: attention for batch b, two local heads ---
                    for hl in range(2):
                        pscore = ps_sm.tile([P, P], F32, name="pscore", tag="pssm")
                        for so in range(16):
                            nc.tensor.matmul(
                                pscore[:], QK[:, so, hl * 128:(hl + 1) * 128],
                                QK[:, so, 256 + hl * 128:256 + (hl + 1) * 128],
                                start=(so == 0), stop=(so == 15),
                            )
                        rowmax = small.tile([P, 1], F32, name="rowmax", tag="rowmax")
                        nc.vector.reduce_max(rowmax[:], pscore[:], axis=mybir.AxisListType.X)
                        negmax = small.tile([P, 1], F32, name="negmax", tag="negmax")
                        nc.vector.tensor_scalar_mul(negmax[:], rowmax[:], -SM_SCALE)
                        wexp = small.tile([P, P], F32, name="wexp", tag="wexp")
                        nc.scalar.activation(wexp[:], pscore[:], AF.Exp,
                                             bias=negmax[:], scale=SM_SCALE)
                        rowsum = small.tile([P, 1], F32, name="rowsum", tag="rowsum")
                        nc.vector.reduce_sum(rowsum[:], wexp[:], axis=mybir.AxisListType.X)
                        rinv = small.tile([P, 1], F32, name="rinv", tag="rinv")
                        nc.vector.reciprocal(rinv[:], rowsum[:])
                        wnorm = small.tile([P, P], BF16, name="wnorm", tag="wnorm")
                        nc.vector.tensor_scalar_mul(wnorm[:], wexp[:], rinv[:])
                        pwt = ps_sm.tile([P, P], BF16, name="pwt", tag="pssm")
                        nc.tensor.transpose(pwt[:], wnorm[:], ident[:])
                        wT = small.tile([P, P], BF16, name="wT", tag="wT")
                        nc.vector.tensor_copy(wT[:], pwt[:])
                        for so in range(16):
                            pat = ps_sm.tile([P, P], F32, name="pat", tag="pssm")
                            nc.tensor.matmul(
                                pat[:], VT[:, hl, so * 128:(so + 1) * 128], wT[:],
                                start=True, stop=True,
                            )
                            nc.vector.tensor_copy(
                                attnT[:, so, b * 256 + hl * 128:b * 256 + (hl + 1) * 128],
                                pat[:])

            # ============ Phase 3: W2 + residual + LN1 stats ============
            with (
                tc.tile_pool(name="t1pool", bufs=1) as t1_pool,
                tc.tile_pool(name="w2pool", bufs=2) as w2_pool,
            ):
                t1T = t1_pool.tile([P, 16, 1024], F32, name="t1T")

                for nb in range(16):
                    w2t = w2_pool.tile([P, 16, 128], BF16, name="w2t", tag="w2t")
                    nc.sync.dma_start(w2t[:], w2[nb])
                    for bp in range(2):
                        pw2 = ps_big.tile([P, 512], F32, name="pw2", tag="psbig")
                        for so in range(16):
                            nc.tensor.matmul(
                                pw2[:], w2t[:, so, :],
                                attnT[:, so, bp * 512:(bp + 1) * 512],
                                start=(so == 0), stop=(so == 15),
                            )
                        xr = w2_pool.tile([P, 512], F32, name="xr", tag="xr")
                        nc.sync.dma_start(xr[:], xres[:, nb, bp * 512:(bp + 1) * 512])
                        t1s = t1T[:, nb, bp * 512:(bp + 1) * 512]
                        nc.vector.tensor_tensor(t1s, pw2[:], xr[:], OP.add)
                        nc.vector.tensor_scalar(t1s, t1s, b2_sb[:, nb:nb + 1], None, OP.add)
                        nc.vector.bn_stats(stats1[:, nb * 2 + bp, :], t1s)

                # ============ Phase 4: LN1 (AllReduce) -> x1 ============
                s1, c1 = _layernorm_scalars(
                    nc, tc, stats, dram, ps_red, ones, epsb, stats1, lnw1_sb, lnb1_sb, "ln1")
                for nb in range(16):
                    nc.vector.tensor_scalar(
                        x1T[:, nb, :], t1T[:, nb, :],
                        s1[:, nb:nb + 1], c1[:, nb:nb + 1], OP.mult, OP.add)

        # ============ Phase 5: FFN ============
        t2_dram = dram.tile([P, 16, 1024], F32, name="t2_dram")
        with (
            tc.tile_pool(name="hpool", bufs=1) as h_pool,
            tc.tile_pool(name="fcpool", bufs=3) as fc_pool,
            tc.tile_pool(name="projpool", bufs=2) as proj_pool,
        ):
            for ch in range(2):
                hT = h_pool.tile([P, 64, 512], BF16, name="hT", tag="hT")
                for fb in range(64):
                    fct = fc_pool.tile([P, 16, 128], BF16, name="fct", tag="fct")
                    nc.sync.dma_start(fct[:], fc[fb])
                    ph = ps_big.tile([P, 512], F32, name="ph", tag="psbig")
                    for do in range(16):
                        nc.tensor.matmul(
                            ph[:], fct[:, do, :],
                            x1T[:, do, ch * 512:(ch + 1) * 512],
                            start=(do == 0), stop=(do == 15),
                        )
                    nc.scalar.activation(hT[:, fb, :], ph[:], AF.Gelu_apprx_tanh,
                                         bias=fcb_sb[:, fb:fb + 1], scale=1.0)
                for nb in range(16):
                    pjt = proj_pool.tile([P, 64, 128], BF16, name="pjt", tag="pjt")
                    nc.sync.dma_start(pjt[:], proj[nb])
                    pm = ps_big.tile([P, 512], F32, name="pm", tag="psbig")
                    for fo in range(64):
                        nc.tensor.matmul(
                            pm[:], pjt[:, fo, :], hT[:, fo, :],
                            start=(fo == 0), stop=(fo == 63),
                        )
                    t2s = proj_pool.tile([P, 512], F32, name="t2s", tag="t2s")
                    nc.vector.tensor_tensor(
                        t2s[:], pm[:], x1T[:, nb, ch * 512:(ch + 1) * 512], OP.add)
                    nc.vector.tensor_scalar(
                        t2s[:], t2s[:], projb_sb[:, nb:nb + 1], None, OP.add)
                    nc.vector.bn_stats(stats2[:, nb * 2 + ch, :], t2s[:])
                    nc.sync.dma_start(t2_dram[:, nb, ch * 512:(ch + 1) * 512], t2s[:])

        # ============ Phase 6: LN2 (AllReduce) -> output ============
        s2, c2 = _layernorm_scalars(
            nc, tc, stats, dram, ps_red, ones, epsb, stats2, lnw2_sb, lnb2_sb, "ln2")
        with tc.tile_pool(name="outpool", bufs=3) as out_pool:
            for nb in range(16):
                t2r = out_pool.tile([P, 1024], F32, name="t2r", tag="t2r")
                nc.sync.dma_start(t2r[:], t2_dram[:, nb, :])
                ys = out_pool.tile([P, 1024], F32, name="ys", tag="ys")
                nc.vector.tensor_scalar(
                    ys[:], t2r[:], s2[:, nb:nb + 1], c2[:, nb:nb + 1], OP.mult, OP.add)
                nc.sync.dma_start(y_out[:, nb, :], ys[:])


def _layernorm_scalars(nc, tc, stats, dram, ps_red, ones, epsb, stats_t, lnw_sb, lnb_sb, tag):
    """From per-tile bn_stats, AllReduce global (mean-ish, meansq-ish) and
    return per-[n_in, n_out] scale/shift tiles s, c with x_norm = t*s + c."""
    mv = stats.tile([P, 2], F32, name=f"mv_{tag}")
    nc.vector.bn_aggr(mv[:], stats_t[:])
    # red_in[:,0] = mean_p ; red_in[:,1] = meansq_p = var_p + mean_p^2
    red_in = stats.tile([P, 2], F32, name=f"red_in_{tag}")
    nc.vector.tensor_copy(red_in[:, 0:1], mv[:, 0:1])
    nc.vector.tensor_tensor(red_in[:, 1:2], mv[:, 0:1], mv[:, 0:1], OP.mult)
    nc.vector.tensor_tensor(red_in[:, 1:2], red_in[:, 1:2], mv[:, 1:2], OP.add)
    pred = ps_red.tile([1, 2], F32, name=f"pred_{tag}", tag="psred")
    nc.tensor.matmul(pred[:], ones[:], red_in[:], start=True, stop=True)
    cc_sb = stats.tile([1, 8], F32, name=f"cc_sb_{tag}")
    nc.vector.memset(cc_sb[:], 0.0)
    nc.vector.tensor_copy(cc_sb[:, 0:2], pred[:])
    cc_in = dram.tile([1, 8], F32, name=f"cc_in_{tag}")
    cc_out = dram.tile([1, 8], F32, name=f"cc_out_{tag}", addr_space="Shared")
    nc.sync.dma_start(cc_in[:], cc_sb[:])
    nc.gpsimd.collective_compute(
        "AllReduce", OP.add,
        replica_groups=[list(range(N_CORES))],
        ins=[cc_in.opt()], outs=[cc_out.opt()],
    )
    g_sb = stats.tile([P, 8], F32, name=f"g_sb_{tag}")
    nc.gpsimd.dma_start(g_sb[:], cc_out[:].to_broadcast((P, 8)))
    # mu = A/1024 ; ex2 = Bq/1024 ; var = (ex2 - mu^2) * N/(N-1) ; rstd = 1/sqrt(var+eps)
    mu = stats.tile([P, 1], F32, name=f"mu_{tag}")
    nc.vector.tensor_scalar_mul(mu[:], g_sb[:, 0:1], 1.0 / N_PGROUPS)
    ex2 = stats.tile([P, 1], F32, name=f"ex2_{tag}")
    nc.vector.tensor_scalar_mul(ex2[:], g_sb[:, 1:2], 1.0 / N_PGROUPS)
    var = stats.tile([P, 1], F32, name=f"var_{tag}")
    nc.vector.tensor_tensor(var[:], mu[:], mu[:], OP.mult)
    nc.vector.tensor_sub(var[:], ex2[:], var[:])
    nc.vector.tensor_scalar_mul(var[:], var[:], N_GLOB / (N_GLOB - 1.0))
    sd = stats.tile([P, 1], F32, name=f"sd_{tag}")
    nc.scalar.activation(sd[:], var[:], AF.Sqrt, bias=epsb[:])
    rstd = stats.tile([P, 1], F32, name=f"rstd_{tag}")
    nc.vector.reciprocal(rstd[:], sd[:])
    s = stats.tile([P, 16], F32, name=f"s_{tag}")
    nc.vector.tensor_scalar_mul(s[:], lnw_sb[:], rstd[:])
    c = stats.tile([P, 16], F32, name=f"c_{tag}")
    nc.vector.tensor_scalar_mul(c[:], s[:], mu[:])
    nc.vector.tensor_sub(c[:], lnb_sb[:], c[:])
    return s, c


# ---------------------------------------------------------------------------
# Host-side input preparation / output gather
# ---------------------------------------------------------------------------

def _bf16(a):
    return np.ascontiguousarray(a.astype(ml_dtypes.bfloat16))


def _f32(a):
    return np.ascontiguousarray(a.astype(np.float32))


def _prep_core_inputs(c, x, W1_w, W1_b, W2_w, W2_b, fc_w, fc_b, proj_w, proj_b,
                      ln1_w, ln1_b, ln2_w, ln2_b):
    r0 = 256 * c
    # x^T tiles [b, sb, d_in, d_out, s]
    xq = _bf16(x.reshape(B, 4, 512, 16, 128).transpose(0, 1, 4, 3, 2))
    # QK weight slice: Q rows [r0:r0+256], K rows [2048+r0 : 2048+r0+256]
    wqk = np.concatenate([W1_w[r0:r0 + 256], W1_w[D + r0:D + r0 + 256]], axis=0)
    w1qk = _bf16(wqk.T.reshape(16, 128, 512).transpose(1, 0, 2))
    bqk = np.concatenate([W1_b[r0:r0 + 256], W1_b[D + r0:D + r0 + 256]])
    b1qk = _f32(np.broadcast_to(bqk[None, :], (P, 512)))
    wv = W1_w[2 * D + r0:2 * D + r0 + 256]
    w1v = _bf16(wv.T.reshape(16, 128, 256).transpose(1, 0, 2))
    b1v = _f32(W1_b[2 * D + r0:2 * D + r0 + 256].reshape(2, 128).T)
    # W2^T tiles [n_blk, s_in, s_out, n]
    w2 = _bf16(W2_w.reshape(16, 128, 16, 128).transpose(0, 3, 2, 1))
    b2 = _f32(W2_b.reshape(16, 128).T)
    # residual rows x^T  [n_in, n_out, i]  i = b*256 + r
    xs = x[:, r0:r0 + 256, :]                       # [4, 256, 2048]
    xres = _f32(xs.transpose(2, 0, 1).reshape(16, 128, 1024).transpose(1, 0, 2))
    fct = _bf16(fc_w.reshape(64, 128, 16, 128).transpose(0, 3, 2, 1))
    fcbt = _f32(fc_b.reshape(64, 128).T)
    projt = _bf16(proj_w.reshape(16, 128, 64, 128).transpose(0, 3, 2, 1))
    projbt = _f32(proj_b.reshape(16, 128).T)
    vec = lambda v: _f32(v.reshape(16, 128).T)
    return {
        "xq": xq, "w1qk": w1qk, "b1qk": b1qk, "w1v": w1v, "b1v": b1v,
        "w2": w2, "b2": b2, "xres": xres, "fc": fct, "fcb": fcbt,
        "proj": projt, "projb": projbt,
        "lnw1": vec(ln1_w), "lnb1": vec(ln1_b),
        "lnw2": vec(ln2_w), "lnb2": vec(ln2_b),
    }


_NC_CACHE = None


def kernel(x, W1_w, W1_b, W2_w, W2_b, fc_w, fc_b, proj_w, proj_b,
           ln1_w, ln1_b, ln2_w, ln2_b):
    global _NC_CACHE, LAST_RESULT
    if TRACE:
        _register_ntff_hook()
    args = [np.asarray(a) for a in
            (x, W1_w, W1_b, W2_w, W2_b, fc_w, fc_b, proj_w, proj_b,
             ln1_w, ln1_b, ln2_w, ln2_b)]
    if _NC_CACHE is None:
        _NC_CACHE = build_program()
    nc = _NC_CACHE
    in_maps = [_prep_core_inputs(c, *args) for c in range(N_CORES)]
    res = bass_utils.run_bass_kernel_spmd(
        nc, in_maps, core_ids=list(range(N_CORES)), trace=TRACE,
    )
    LAST_RESULT = res
    out = np.empty((B, S, D), np.float32)
    for c in range(N_CORES):
        yt = res.results[c]["y"]                    # [128 n_in, 16 n_out, 1024 i]
        blk = yt.reshape(128, 16, 4, 256).transpose(2, 3, 1, 0).reshape(4, 256, D)
        out[:, 256 * c:256 * (c + 1), :] = blk
    return out
